# revision 27
# baseline (speedup 1.0000x reference)
# Distributed causal multi-head attention kernel for one TRN2 chip (8 NeuronCores).
#
# Problem: x[2, 2048, 1024], 16 heads, head_dim 64, causal, MASK_VAL=-50000.
#   out = softmax(causal(q k^T / 8)) v @ Wo  with q = x Wq, (k|v) = x Wkv.
#
# Sharding (batch+head): core c handles batch c//4 and the 4 heads
# (c%4)*4 .. +4 (Wq/Wkv column-parallel, Wo row-parallel).  Each core writes
# a partial [2048, 1024] output; the host sums the 4 partials per batch.
# No on-device collectives.
#
# Per-core layout strategy (all bf16 compute, f32 PSUM accumulate):
#   host feeds xT = x[b].T  -> projections need no on-device transpose:
#     qT[hd,n] = Wq_shard.T @ x.T : matmul(lhsT=Wq, rhs=xT)
#     kT[hd,n] likewise; v[n,hd] = matmul(lhsT=xT, rhs=Wv)
#   scoresT[j,i] = matmul(lhsT=kT block, rhs=qT block)   (K=hd=64)
#     - even/odd heads of a pair live at partitions 0:64 / 64:128 so their
#       K=64 matmuls land in different PE row groups and run concurrently.
#   softmax: no max subtraction needed (scores ~ N(0,1); exp(-50000) == 0.0
#     in f32 exactly, matching the reference's masked softmax).  exp on ACT
#     with scale=1/8 fused.  Row sums come for free: v is augmented with a
#     ones column, so PV matmul row 64 (even head) / 32 (odd head)
#     accumulates sum_j exp.
#   causal: fully-masked j-blocks skipped; diagonal blocks compute only the
#     live column range and apply a 128x128 triangular 0/1 mask (host input).
#   out = matmul(lhsT=outT, rhs=Wo_shard), streamed out per 128-row chunk
#     in bf16 (host accumulates partials in f32).
#
# v3 perf notes (driven by the ntff profile of v2 @ 170us; engine-busy:
# PE 144us, ACT/exp 97us, DVE 89us over a 176us span -> the kernel is
# co-paced by PE and ACT, with ~30us lost in the tail and ~12us at start):
#   - ATTENTION PHASE IS ACT(exp)-PACED in the last i-chunk and PE-paced in
#     the first chunks; all Wo work is scheduled into chunks 2-3 (ACT-bound)
#     and the tail, where the PE has slack.
#   - the v2 reciprocal DMA "dance" (gather->DVE recip->scatter, ~10us
#     round-trip) is replaced FOR THE LAST PAIR by two ACT passes:
#     1/d = Exp(-Ln(d)), reading the denominators straight from PSUM.  Both
#     functions live in the same ACT table set (natural_log_exp_and_others)
#     so there is no mid-kernel table reload.  Wo chunks 8-11 (whose oT rows
#     were normalized one chunk earlier) are deferred to run DURING that
#     latency, keeping the PE busy and the HAM clock warm - v2 burned this
#     window on heater matmuls and still went cold (13.6us at K=4/8).
#   - v_sb zero-memset removed (6.9us DVE at startup): the pad columns of
#     the padded-to-128 v tiles only feed PSUM rows that are never read, so
#     junk SBUF is harmless; only the two ones-columns are memset.
#   - xT quarters are split kc 0:4 / 4:8 across the scalar and gpsimd DMA
#     rings (two HWDGE queues spin up in parallel -> first quarter lands
#     ~3us earlier); weights ride the sync ring with the pair-0 slices of
#     Wq/Wk first (host stores them pair-major) so the first projection
#     only waits on 0.5MB + xT q0.
#   - output DMAs round-robin sync/gpsimd mid-kernel and sync/scalar/gpsimd
#     in the tail drain.

import numpy as np
import ml_dtypes

import concourse.bass as bass
import concourse.mybir as mybir
import concourse.tile as tile
from concourse.bass_utils import run_bass_kernel_spmd


def _install_axon_ntff_shim():
    """This container's `antenv` lacks `axon_hooks`, which bass_utils imports
    when tracing under axon.  Provide the module and install the ctypes NTFF
    hook against libaxon_pjrt.so so BASS_TRACE=1 profiling works."""
    import sys
    import types
    import contextlib
    import ctypes
    try:
        import antenv.axon_hooks  # noqa: F401
        return
    except ImportError:
        pass
    try:
        import antenv
    except ImportError:
        return
    mod = types.ModuleType("antenv.axon_hooks")
    state = {"hook": None}
    mod.set_axon_ntff_profile_hook = lambda h: state.__setitem__("hook", h)
    mod.get_axon_ntff_profile_hook = lambda: state["hook"]
    sys.modules["antenv.axon_hooks"] = mod
    antenv.axon_hooks = mod
    so_path = "/opt/axon/libaxon_pjrt.so"
    try:
        lib = ctypes.CDLL(so_path)
        if not hasattr(lib, "axon_start_nrt_profile"):
            return
        lib.axon_start_nrt_profile.argtypes = [
            ctypes.POINTER(ctypes.c_int64), ctypes.c_size_t]
        lib.axon_start_nrt_profile.restype = ctypes.c_int64
        lib.axon_stop_nrt_profile.argtypes = [ctypes.c_char_p]
        lib.axon_stop_nrt_profile.restype = ctypes.c_int64

        @contextlib.contextmanager
        def _hook(output_dir, device_ids):
            import jax
            jax.devices()
            if device_ids:
                ids = (ctypes.c_int64 * len(device_ids))(*device_ids)
                rc = lib.axon_start_nrt_profile(ids, len(device_ids))
            else:
                rc = lib.axon_start_nrt_profile(None, 0)
            if rc != 0:
                raise RuntimeError(f"axon_start_nrt_profile rc={rc}")
            try:
                yield
            finally:
                n = lib.axon_stop_nrt_profile(str(output_dir).encode())
                print(f"ntff profile: {n} file(s) -> {output_dir}")

        mod.set_axon_ntff_profile_hook(_hook)
    except Exception:
        pass


_install_axon_ntff_shim()

BF16 = ml_dtypes.bfloat16
P = 128
N = 2048          # sequence length
D = 1024          # model dim
HD = 64           # head dim
HL = 4            # local heads per core
DQ = HL * HD      # 256 local projection width
KC = D // P       # 8 contraction chunks
NPAIR = HL // 2   # head pairs (even@part 0:64, odd@part 64:128)
IC = 512          # i-chunk (query) width
NIC = N // IC     # 4
NJB = N // P      # 16 j-blocks
F32 = mybir.dt.float32
BF = mybir.dt.bfloat16

LAST_RESULT = {}


def build_nc():
    nc = bass.Bass()
    # host pre-reshapes every input so each DMA slice is contiguous:
    #   xT:  x[b].T as [128, 4 i-quarters, 8 kc, 512]  (8 half-quarter DMAs)
    #   wq/wk: [128, 2 pairs, 8 kc, 128] (pair-major), wv: [128, 8, 256],
    #   wo: [128, 2, 1024]
    xT = nc.declare_dram_parameter("xT", [P, NIC, KC, IC], BF, isOutput=False)
    wq = nc.declare_dram_parameter("wq", [P, NPAIR, KC, P], BF, isOutput=False)
    wk = nc.declare_dram_parameter("wk", [P, NPAIR, KC, P], BF, isOutput=False)
    wv = nc.declare_dram_parameter("wv", [P, KC, DQ], BF, isOutput=False)
    wo = nc.declare_dram_parameter("wo", [P, 2, D], BF, isOutput=False)
    mask = nc.declare_dram_parameter("mask", [P, 2, P], BF, isOutput=False)
    out = nc.declare_dram_parameter("out", [N, D], BF, isOutput=True)

    Exp = mybir.ActivationFunctionType.Exp
    Ln = mybir.ActivationFunctionType.Ln

    with tile.TileContext(nc) as tc:
        with (
            tc.tile_pool(name="const", bufs=1) as constp,
            tc.tile_pool(name="expp", bufs=8) as expp,
            tc.tile_pool(name="normp", bufs=4) as normp,
            tc.tile_pool(name="outp", bufs=3) as outp,
            tc.tile_pool(name="psS", bufs=2, space="PSUM") as psS,
            tc.tile_pool(name="psO", bufs=1, space="PSUM") as psO,
            tc.tile_pool(name="psM", bufs=2, space="PSUM") as psM,
        ):
            # ---------------- resident SBUF tensors + input DMA ----------------
            # Weights ride the sync ring, pair-0 slices first.  xT quarters
            # are split kc 0:4 / 4:8 across the scalar and gpsimd rings so
            # both HWDGE queues spin up in parallel and quarter 0's ~1MB
            # lands in half the time.
            wq_sb = constp.tile([P, NPAIR, KC, P], BF, tag="wq")
            wk_sb = constp.tile([P, NPAIR, KC, P], BF, tag="wk")
            wv_sb = constp.tile([P, KC, DQ], BF, tag="wv")
            xT_sb = constp.tile([P, NIC, KC, IC], BF, tag="xT")
            # Ring assignment = strict need order.  Three rings spin up in
            # parallel; the first projection needs wq0 + xT q0 (~12us), the
            # pair-1 projections follow ~2us later, so the pair-1 weights
            # ride right behind the q0 halves on the xT rings rather than
            # queueing behind wv on the sync ring.
            nc.sync.dma_start(wq_sb[:, 0], wq[:, 0])
            nc.sync.dma_start(wk_sb[:, 0], wk[:, 0])
            # quarter 0 split kc 0:4 / 4:8 across two rings (finer splits
            # lose more to ~600ns per-DMA queue-issue than they gain)
            nc.scalar.dma_start(xT_sb[:, 0, 0:KC // 2, :],
                                xT[:, 0, 0:KC // 2, :])
            nc.gpsimd.dma_start(xT_sb[:, 0, KC // 2:KC, :],
                                xT[:, 0, KC // 2:KC, :])
            nc.scalar.dma_start(wq_sb[:, 1], wq[:, 1])
            nc.gpsimd.dma_start(wk_sb[:, 1], wk[:, 1])
            for q in range(1, NIC):
                nc.scalar.dma_start(xT_sb[:, q, 0:KC // 2, :],
                                    xT[:, q, 0:KC // 2, :])
                nc.gpsimd.dma_start(xT_sb[:, q, KC // 2:KC, :],
                                    xT[:, q, KC // 2:KC, :])
            nc.sync.dma_start(wv_sb[:], wv[:, :, :])
            mask_sb = constp.tile([P, 2, P], BF, tag="mask")
            nc.sync.dma_start(mask_sb[:], mask[:, :, :])
            wo_sb = constp.tile([P, 2, D], BF, tag="wo")
            nc.sync.dma_start(wo_sb[:], wo[:, :, :])

            qT_sb = constp.tile([P, NPAIR, N], BF, tag="qT")
            kT_sb = constp.tile([P, NPAIR, N], BF, tag="kT")
            # heater operand first: zeros, so the dead matmuls that keep the
            # PE busy through the HAM cold window write benign values, and
            # the heater can start as early as possible.
            heat_sb = constp.tile([P, IC], BF, tag="heat")
            nc.vector.memset(heat_sb[:], 0.0)
            # v, head-major, PADDED TO 128 WEIGHT COLUMNS per head.  EVEN
            # heads: [v(64) | ones@64 | pad] — PV output rows 0:64, the
            # denominator (sum_j exp) accumulates in row 64.  ODD heads:
            # [pad | ones@32 | pad | v@64:128] — PV output lands DIRECTLY at
            # partitions 64:128 (where the Wo matmul needs it; DVE is
            # lane-locked, so producing it anywhere else would cost an
            # SBUF->SBUF partition-shift DMA on the critical tail), with the
            # denominator in row 32.  The pad columns are NEVER memset: they
            # only feed PSUM rows that are never read, so junk is harmless
            # (v2 spent 6.9us of DVE zeroing them).
            v_sb = constp.tile([P, NJB, HL, P], BF, tag="v")
            oT_sb = constp.tile([P, NPAIR, N], BF, tag="oT")
            nc.vector.memset(v_sb[:, :, 0:HL:2, HD], 1.0)
            nc.vector.memset(v_sb[:, :, 1:HL:2, 32], 1.0)
            # bf16 ones rows for the reciprocal-broadcast outer products
            # (lhsT/rhs of a K=1 matmul must share a base partition, and bass
            # only allows base 0/32/64).  Row 0/32 serve the DVE dance path;
            # rows 64/32 serve the ACT Ln/Exp path of the final normalize
            # (whose recips live at the PSUM denominator rows 64/32).
            ones_sb = constp.tile([P, HD], BF, tag="ones")
            nc.vector.memset(ones_sb[0:1, :], 1.0)
            nc.vector.memset(ones_sb[32:33, :], 1.0)
            nc.vector.memset(ones_sb[64:65, :], 1.0)

            # ---------------- PE heater ----------------
            # Dead matmuls keep the PE continuously busy from kernel start
            # until the first projection's inputs have landed (~12us: 6.6us
            # framework preamble + HWDGE spin-up + 1MB xT quarter 0).  HAM
            # only un-throttles the PE clock to 2.4 GHz after a fully-busy
            # free-running 4096-cycle window, so the busy streak must be
            # GAP-FREE from heater into projections.
            for _ in range(20):
                hp = psM.tile([HD, IC], F32, tag="mm", name="heat")
                nc.tensor.matmul(
                    hp[:, 0:IC // 2], heat_sb[:, 0:HD], heat_sb[:, 0:IC // 2],
                    start=True, stop=True,
                )

            # ---------------- projections ----------------
            def proj_qk(w_sb, dst, pair, i4):
                ps = psM.tile([P, IC], F32, tag="mm")
                for kc in range(KC):
                    nc.tensor.matmul(
                        ps[:],
                        w_sb[:, pair, kc, :],
                        xT_sb[:, i4, kc, :],
                        start=(kc == 0), stop=(kc == KC - 1),
                    )
                nc.vector.tensor_copy(dst[:, pair, i4 * IC:(i4 + 1) * IC], ps[:])

            def proj_v(jc):
                ps = psM.tile([P, IC], F32, tag="mm")
                for kc in range(KC):
                    nc.tensor.matmul(
                        ps[:, :DQ],
                        xT_sb[:, jc // 4, kc, (jc % 4) * P:(jc % 4 + 1) * P],
                        wv_sb[:, kc, :],
                        start=(kc == 0), stop=(kc == KC - 1),
                    )
                vh = ps[:, :DQ].rearrange("p (h e) -> p h e", e=HD)
                nc.vector.tensor_copy(v_sb[:, jc, 0:HL:2, 0:HD], vh[:, 0:HL:2, :])
                nc.vector.tensor_copy(v_sb[:, jc, 1:HL:2, HD:P], vh[:, 1:HL:2, :])

            # ---------------- attention (+ interleaved Wo) ----------------
            _wo_ring = [0]

            def wo_chunk(mc, evac="vv", use_psO=False,
                         rings=(nc.sync, nc.gpsimd)):
                # output rows mc*128..+128, all 1024 cols, bf16 out.
                # use_psO: in the tail the PV accumulator banks are dead, so
                # alternate chunks borrow them — two psum tiles in flight
                # keep the Wo matmul stream at full pace (psM alone rotates
                # at evacuation pace, which starved the PE into a HAM
                # re-throttle).  evac[half]: 'v' = DVE cast, 's' = ACT copy
                # — the tail splits evacuations across both engines because
                # either one alone is slower than the matmul stream.
                osb = outp.tile([P, D], BF, tag="osb")
                pso = psO.tile([P, 2, IC], F32, tag="po",
                               name="wops") if use_psO else None
                for half in range(2):
                    ps = pso[:, half] if use_psO else psM.tile(
                        [P, IC], F32, tag="mm")
                    for kc2 in range(2):
                        nc.tensor.matmul(
                            ps[:],
                            oT_sb[:, kc2, mc * P:(mc + 1) * P],
                            wo_sb[:, kc2, half * IC:(half + 1) * IC],
                            start=(kc2 == 0), stop=(kc2 == 1),
                        )
                    if evac[half] == "s":
                        nc.scalar.copy(osb[:, half * IC:(half + 1) * IC], ps[:])
                    else:
                        nc.vector.tensor_copy(
                            osb[:, half * IC:(half + 1) * IC], ps[:])
                    # per-half output DMA: the first half streams out while
                    # the second half's matmuls run; round-robin across the
                    # given rings for a shorter drain
                    q = rings[_wo_ring[0] % len(rings)]
                    _wo_ring[0] += 1
                    q.dma_start(
                        out[mc * P:(mc + 1) * P, half * IC:(half + 1) * IC],
                        osb[:, half * IC:(half + 1) * IC],
                    )

            # Wo fillers all land in i-chunks 2-3: the early chunks are
            # PE-bound (they carry the next chunk's projections), while
            # chunks 2-3 are exp(ACT)-paced with PE slack.  Chunks 8-15 are
            # handled in the tail: 8-11 cover the final normalize's ACT
            # round-trip with real PE work (v2 used heater matmuls and still
            # went HAM-cold there), 12-15 follow the final normalize.
            wo_for_ic = {3: [0, 1]}

            # ---- flat attention stream with cross-boundary pipelining ----
            # The scores->exp pipeline runs 2 units ahead of the PV stream
            # and flows ACROSS pair and i-chunk boundaries, so the first PV
            # of a new pair overlaps the previous pair's tail PVs + psum
            # evacuation.
            units = []
            for i4 in range(NIC):
                nb = 4 * i4 + 4
                for pair in range(NPAIR):
                    for jb in range(nb):
                        units.append((i4, pair, jb, nb))

            eTs = {}
            po2 = {}

            # Chunks 2-3 are exp(ACT)-paced while the PE and DVE have slack
            # there, so a subset of their full (non-diagonal) j-blocks
            # compute exp on the DVE instead: a Schraudolph approximation in
            # bf16 bit space — round(s * 16*log2(e) + (127*2^7 - 7)) written
            # as int16 IS the bf16 bit pattern of ~exp(s/8) (rms err ~1.8%,
            # zero mean; the softmax renormalizes, and the affected rows are
            # a minority, so the output penalty is a few 1e-3 against a 2e-2
            # budget).  One tensor_scalar per unit, reading the score PSUM
            # directly.
            SCH_A = 16.0 * 1.4426950408889634
            SCH_B = 127.0 * 128.0 - 7.0 + 0.25
            I16 = mybir.dt.int16
            sch_units = {(2, 3), (2, 7), (3, 1), (3, 3), (3, 5), (3, 7),
                         (3, 9)}

            def scores_exp(i4, pair, jb):
                r = jb - 4 * i4  # >=0 -> diagonal block
                lo = max(0, r * P)
                pss = psS.tile([P, 2, IC], F32, tag="pss", name="pss")
                for h01 in range(2):
                    pb = h01 * HD
                    nc.tensor.matmul(
                        pss[:, h01, lo:IC],
                        kT_sb[pb:pb + HD, pair, jb * P:(jb + 1) * P],
                        qT_sb[pb:pb + HD, pair, i4 * IC + lo:(i4 + 1) * IC],
                        start=True, stop=True,
                    )
                eT = expp.tile([P, 2, IC], BF, tag="eT", name="eT")
                if (i4, jb) in sch_units:
                    nc.vector.tensor_scalar(
                        eT[:, :, :].bitcast(I16), pss[:, :, :],
                        SCH_A, SCH_B,
                        mybir.AluOpType.mult, mybir.AluOpType.add,
                    )
                else:
                    nc.scalar.activation(
                        eT[:, :, lo:IC], pss[:, :, lo:IC], Exp, scale=0.125
                    )
                if r >= 0:
                    nc.vector.tensor_mul(
                        eT[:, :, lo:lo + P], eT[:, :, lo:lo + P], mask_sb[:]
                    )
                eTs[(i4, pair, jb)] = (eT, lo)

            def pv(i4, pair, jb, nb):
                if jb == 0:
                    # both heads' accumulators in ONE 2-bank psum tile so the
                    # evacuation is a single DVE cast
                    po2[pair] = psO.tile([P, 2, IC], F32, tag="po",
                                         name=f"po{pair}")
                eT, lo = eTs.pop((i4, pair, jb))
                for h01 in range(2):
                    nc.tensor.matmul(
                        po2[pair][:, h01, lo:IC],
                        v_sb[:, jb, 2 * pair + h01, :],
                        eT[:, h01, lo:IC],
                        start=(jb == 0), stop=(jb == nb - 1),
                    )

            def normalize_start(i4, pair):
                # head 0: PV in po rows 0:64, exp-sum in row 64.
                # head 1: PV in po rows 64:128, exp-sum in row 32.
                # Reciprocal via the DVE dance: the 2x512 sums live on ONE
                # partition each and DVE reciprocal is an 8-pass iterative
                # divide (~4us serial there), so gather the sums over 64
                # partitions via SBUF->SBUF DMA, reciprocate in parallel
                # lanes, and scatter back to rows 0 (head 0) / 32 (head 1).
                # The ~5us round-trip hides under the next pair's attention;
                # the PE-side finish (broadcast + muls) is emitted several
                # units later so the in-order PE queue never stalls on it.
                po = po2.pop(pair)
                posb = normp.tile([P, 2, IC], BF, tag="posb")
                # h1 (denominator included — DVE cost is free-dim-bound)
                # evacuated first so its denominator's gather DMA flies
                # earliest.
                nc.vector.tensor_copy(posb[:, 1, :], po[:, 1, :])
                with nc.allow_low_precision(
                        "softmax denominators are well-conditioned"):
                    sT2 = normp.tile([HD, 16], BF, tag="sT")
                    nc.sync.dma_start(sT2[32:HD, :], posb[32:33, 1, :])
                    nc.vector.tensor_copy(
                        posb[0:HD + 1, 0, :], po[0:HD + 1, 0, :])
                    nc.sync.dma_start(sT2[0:32, :], posb[HD:HD + 1, 0, :])
                    rT2 = normp.tile([HD, 16], BF, tag="rT")
                    nc.vector.reciprocal(rT2[:], sT2[:])
                    rc2 = normp.tile([P, IC], BF, tag="rc")
                    nc.sync.dma_start(rc2[0:1, :], rT2[0:32, :])
                    nc.sync.dma_start(rc2[32:33, :], rT2[32:HD, :])
                return posb, rc2

            def normalize_finish(i4, pair, posb, rc2):
                # broadcast each recip row across 64 partitions via K=1
                # outer products on PE (share the psM "mm" slots); head 1's
                # broadcast targets psum partitions 64:128.
                bc0 = psM.tile([HD, IC], F32, tag="mm", name="bc")
                nc.tensor.matmul(
                    bc0[:], ones_sb[0:1, 0:HD], rc2[0:1, :],
                    start=True, stop=True,
                )
                bc1 = psM.tile([P, IC], F32, tag="mm", name="bc")
                nc.tensor.matmul(
                    bc1[HD:P, :], ones_sb[32:33, 0:HD], rc2[32:33, :],
                    start=True, stop=True,
                )
                nc.vector.tensor_mul(
                    oT_sb[HD:P, pair, i4 * IC:(i4 + 1) * IC],
                    posb[HD:P, 1, :], bc1[HD:P, :],
                )
                nc.vector.tensor_mul(
                    oT_sb[0:HD, pair, i4 * IC:(i4 + 1) * IC],
                    posb[0:HD, 0, :], bc0[:],
                )

            def normalize_last(i4, pair):
                # Final normalize: the exp stream is finished, so ACT is free
                # and the reciprocal runs as 1/d = Exp(-Ln(d)) straight from
                # the PSUM denominator rows — no DMA round-trip.  Ln output
                # must be f32 (a bf16 ln would cost ~3% on the recip).  The
                # Wo chunks 8-11 are emitted between the last PV and the
                # broadcast matmuls, so the PE chews real work (and stays
                # HAM-warm) through the ACT latency.
                po = po2.pop(pair)
                lnd = normp.tile([P, IC], F32, tag="lnd")
                nc.scalar.activation(lnd[64:65, :], po[64:65, 0, :], Ln)
                nc.scalar.activation(lnd[32:33, :], po[32:33, 1, :], Ln)
                rcp = normp.tile([P, IC], BF, tag="rcp")
                nc.scalar.activation(rcp[64:65, :], lnd[64:65, :], Exp,
                                     scale=-1.0)
                nc.scalar.activation(rcp[32:33, :], lnd[32:33, :], Exp,
                                     scale=-1.0)
                # denominator-slice evacuations on ACT too: the DVE is the
                # tail's scarce engine (it still owes the Wo casts + muls)
                posb = normp.tile([P, 2, IC], BF, tag="posb")
                nc.scalar.copy(posb[:, 1, :], po[:, 1, :])
                nc.scalar.copy(posb[0:HD, 0, :], po[0:HD, 0, :])

                # PE work to cover the ACT latency: Wo row-chunks 10-11
                # (8-9 were already threaded into the closing PV units).
                wo_chunk(10, evac="vs", use_psO=True)
                wo_chunk(11, evac="vs")

                # broadcast: h0 recip lives at partition 64 -> out rows 0:64;
                # h1 recip at partition 32 -> out rows 64:128.
                bc0 = psM.tile([HD, IC], F32, tag="mm", name="bc")
                nc.tensor.matmul(
                    bc0[:], ones_sb[64:65, 0:HD], rcp[64:65, :],
                    start=True, stop=True,
                )
                bc1 = psM.tile([P, IC], F32, tag="mm", name="bc")
                nc.tensor.matmul(
                    bc1[HD:P, :], ones_sb[32:33, 0:HD], rcp[32:33, :],
                    start=True, stop=True,
                )
                # normalize in 128-col slices so each closing Wo chunk can
                # start as soon as its columns are scaled
                for s in range(4):
                    sl = slice(s * P, (s + 1) * P)
                    osl = slice(i4 * IC + s * P, i4 * IC + (s + 1) * P)
                    nc.vector.tensor_mul(
                        oT_sb[HD:P, pair, osl], posb[HD:P, 1, sl],
                        bc1[HD:P, sl],
                    )
                    nc.vector.tensor_mul(
                        oT_sb[0:HD, pair, osl], posb[0:HD, 0, sl],
                        bc0[:, sl],
                    )

            def make_fillers(i4):
                # PE filler work paced into this i-chunk's attention stream:
                # the next chunk's projections and deferred Wo chunks.
                fs = []
                if i4 + 1 < NIC:
                    for pair in range(NPAIR):
                        fs.append(
                            lambda p=pair, i=i4 + 1: proj_qk(wq_sb, qT_sb, p, i))
                        fs.append(
                            lambda p=pair, i=i4 + 1: proj_qk(wk_sb, kT_sb, p, i))
                    for jc in range(4 * (i4 + 1), 4 * (i4 + 1) + 4):
                        fs.append(lambda j=jc: proj_v(j))
                for w4 in wo_for_ic.get(i4, []):
                    for mc in range(4 * w4, 4 * w4 + 4):
                        fs.append(lambda m=mc: wo_chunk(m))
                return fs

            # pre-phase: all of chunk 0's projections; the first two
            # scores_exp are emitted right after their kT slice exists so
            # the exp stream warms up during the v projections.
            proj_qk(wq_sb, qT_sb, 0, 0)
            proj_qk(wk_sb, kT_sb, 0, 0)
            proj_qk(wq_sb, qT_sb, 1, 0)
            proj_qk(wk_sb, kT_sb, 1, 0)
            scores_exp(*units[0][:3])
            scores_exp(*units[1][:3])
            for jc in range(4):
                proj_v(jc)

            cur_i4 = -1
            fillers = []
            pending_fin = []  # (due_idx, finish_args)
            fi = it = n_slots = 0
            for idx, (i4, pair, jb, nb) in enumerate(units):
                if i4 != cur_i4:
                    while fi < len(fillers):  # previous chunk's leftovers
                        fillers[fi]()
                        fi += 1
                    cur_i4 = i4
                    fillers = make_fillers(i4)
                    fi = it = 0
                    n_slots = NPAIR * nb
                while pending_fin and pending_fin[0][0] <= idx:
                    normalize_finish(*pending_fin.pop(0)[1])
                if idx + 2 < len(units):
                    scores_exp(*units[idx + 2][:3])
                pv(i4, pair, jb, nb)
                if idx == len(units) - 4:
                    # end-of-stream bridge: the closing diagonal units are
                    # exp-latency-paced with no filler work left; thread the
                    # first two deferred Wo chunks (their oT rows were
                    # normalized a chunk ago) into those holes — real work
                    # that also keeps the HAM clock warm into the tail.
                    wo_chunk(8)
                elif idx == len(units) - 2:
                    wo_chunk(9)
                it += 1
                # proportional pacing: spread fillers evenly over the
                # i-chunk's attention iterations
                while fi < len(fillers) and fi * n_slots <= it * len(fillers):
                    fillers[fi]()
                    fi += 1
                if jb == nb - 1:
                    if idx == len(units) - 1:
                        while fi < len(fillers):
                            fillers[fi]()
                            fi += 1
                        while pending_fin:
                            normalize_finish(*pending_fin.pop(0)[1])
                        normalize_last(i4, pair)
                    else:
                        posb, rc2 = normalize_start(i4, pair)
                        pending_fin.append(
                            (idx + 6, (i4, pair, posb, rc2)))
            # last i-chunk's Wo: evacuations alternate DVE/ACT, DMAs over
            # three rings for the final drain
            for mc in range(12, 16):
                wo_chunk(mc, evac="sv", use_psO=(mc % 2 == 0),
                         rings=(nc.sync, nc.scalar, nc.gpsimd))
    return nc


_LEGALIZE_TYPES = None


def _legalize_pe_waits(nc, max_waits=1):
    """walrus' TPB instruction encodings fit very few semaphore waits
    (Matmult: 1; TensorTensor etc. similarly limited) but Tile sometimes
    emits more.  Move the excess onto an InstNoOp inserted just before the
    instruction in the same engine stream — waiting earlier on the same
    engine is always safe."""
    global _LEGALIZE_TYPES
    if _LEGALIZE_TYPES is None:
        _LEGALIZE_TYPES = (
            mybir.InstMatmult, mybir.InstLdweights, mybir.InstTensorTensor,
            mybir.InstTensorCopy, mybir.InstActivation, mybir.InstReciprocal,
            mybir.InstMemset, mybir.InstTensorReduce, mybir.InstIota,
            mybir.InstTensorScalarPtr, mybir.InstISA, mybir.InstDMACopy,
            mybir.InstTensorTensorReduce, mybir.InstDrain,
            mybir.InstDmaTransposeAnt,
        )
    n_fixed = 0
    for fn in nc.m.functions:
        for blk in fn.blocks:
            insts = list(blk.instructions)
            out = []
            for inst in insts:
                si = getattr(inst, "sync_info", None)
                if (
                    isinstance(inst, _LEGALIZE_TYPES)
                    and si is not None
                    and si.on_wait
                    and len(si.on_wait) > max_waits
                ):
                    extra = list(si.on_wait[:-max_waits])
                    keep = list(si.on_wait[-max_waits:])
                    for w in extra:
                        out.append(mybir.InstEventSemaphore(
                            name=nc.get_next_instruction_name(),
                            engine=inst.engine,
                            ins=[],
                            outs=[],
                            sync_info=mybir.SyncInfo(on_wait=[w], on_update=[]),
                            bass_nofuse=True,
                        ))
                    inst.sync_info = mybir.SyncInfo(
                        on_wait=keep, on_update=list(si.on_update)
                    )
                    n_fixed += 1
                out.append(inst)
            blk.instructions = out
    return n_fixed


_NC_CACHE = {}


def _get_nc():
    if "nc" not in _NC_CACHE:
        nc = build_nc()
        _legalize_pe_waits(nc)
        _NC_CACHE["nc"] = nc
    return _NC_CACHE["nc"]


def _make_mask():
    tri = np.triu(np.ones((P, P), np.float32))  # keep j<=c
    return np.ascontiguousarray(
        np.broadcast_to(tri[:, None, :], (P, 2, P))
    ).astype(BF16)


def _chunk_rows(a, kc):
    # [R, C] -> [128, R//128, C] partition-major (one contiguous DMA on device)
    r, c = a.shape
    return np.ascontiguousarray(
        a.reshape(kc, P, c).transpose(1, 0, 2)
    )


def _chunk_rows_pairs(a):
    # [1024, 256] -> [128, 2 pairs, 8 kc, 128] so each pair's slice is one
    # contiguous DMA
    return np.ascontiguousarray(
        a.reshape(KC, P, NPAIR, P).transpose(1, 2, 0, 3)
    )


def kernel(x, Wq, Wkv, Wo, **kw):
    x = np.asarray(x, np.float32)
    Wq = np.asarray(Wq, np.float32)
    Wkv = np.asarray(Wkv, np.float32)
    Wo = np.asarray(Wo, np.float32)
    mask = _make_mask()

    in_maps = []
    for c in range(8):
        b = c // 4
        hs = (c % 4) * DQ
        xTb = np.ascontiguousarray(x[b].T).astype(BF16)  # [1024, 2048]
        # -> [128, 4 quarters, 8 kc, 512]
        xTr = np.ascontiguousarray(
            xTb.reshape(KC, P, NIC, IC).transpose(1, 2, 0, 3)
        )
        in_maps.append({
            "xT": xTr,
            "wq": _chunk_rows_pairs(Wq[:, hs:hs + DQ].astype(BF16)),
            "wk": _chunk_rows_pairs(Wkv[:, hs:hs + DQ].astype(BF16)),
            "wv": _chunk_rows(Wkv[:, D + hs:D + hs + DQ].astype(BF16), KC),
            "wo": _chunk_rows(Wo[hs:hs + DQ, :].astype(BF16), 2),
            "mask": mask,
        })

    res = run_bass_kernel_spmd(_get_nc(), in_maps, core_ids=list(range(8)))
    LAST_RESULT["exec_time_ns"] = res.exec_time_ns
    LAST_RESULT["trace"] = res.instructions_and_trace
    parts = [np.asarray(r["out"], np.float32) for r in res.results]
    out = np.stack(
        [parts[0] + parts[1] + parts[2] + parts[3],
         parts[4] + parts[5] + parts[6] + parts[7]], axis=0
    )
    return out
